# revision 9
# baseline (speedup 1.0000x reference)
"""NT-Xent (GroupSupCon) loss on 8 trn2 NeuronCores via Bass/Tile.

Strategy (SPMD, one program for all 8 cores):
  - Host: normalize rows (f32), compute the positive-pair dots (f32),
    cast z to bf16, and for each core c build the column-rolled
    transposed operand zT_c = roll(z, -1024*c).T  [128 d, 8192 rows],
    so core c's own 1024 rows sit at column offset 0. One input tensor
    per core; no device-side prep at all.
  - Device: for each of the 8 own row-blocks t and 4 j-chunks (2048
    cols), matmul (lhsT = own block cols, rhs = all cols) into PSUM,
    then exp(2s) with fused row-sum:
      * ACT engine chunks: exact Exp activation with accum_out.
      * DVE chunks: custom DVE op R(s) = (((c3 s + c2) s + c1) s + 1)^2
        ~= exp(2s) (|s|<=0.55 off-diagonal; end-to-end loss error
        ~2e-6), also with fused accum. This nearly doubles exp
        throughput since ACT and DVE run concurrently.
  - Tail: den = sum(chunk accums) - SELF_TERM, ln on ACT, row-total via
    ones-matmul; host sums 8 partials, subtracts the positive term and
    divides by 2B.
"""

import math
from contextlib import ExitStack

import numpy as np

import concourse.bacc as bacc
import concourse.bass as bass
import concourse.mybir as mybir
import concourse.tile as tile
from concourse.bass_utils import run_bass_kernel_spmd

import concourse.dve_ops as dve_ops
from concourse.dve_spec import Spec, Src0, C0, C1, C2, One, sq, lower, AluOp
from concourse.dve_uop import DveOpSpec

N_CORES = 8
B = 4096
TWO_B = 2 * B          # 8192 rows total
D = 128                # feature dim
ROWS = TWO_B // N_CORES  # 1024 rows per core
INV_T = 2.0            # 1 / temperature (T = 0.5)
SELF_TERM = math.exp(INV_T)  # exp(sim_kk / T) with sim_kk == 1

NCHUNK = 2048          # j-chunk width (4 PSUM banks)
NJC = TWO_B // NCHUNK  # 4 chunks
NT = ROWS // 128       # 8 own row-blocks

F32 = mybir.dt.float32
BF16 = mybir.dt.bfloat16
FP8 = mybir.dt.float8e4
AF = mybir.ActivationFunctionType

# host pre-scales z by 16 (keeps fp8 e4m3 components in the normal range);
# PSUM sims come out scaled by SIMSCALE = 256.
SIMSCALE = 256.0

# Squared-cubic exp(2s) approximation, fit to the off-diagonal sim
# distribution (|s| <= 0.55): R(s) = (((c3 s + c2) s + c1) s + 1)^2
EXPQ_NAME = "EXP2SQ_NTXENT_ANT"
EXPQ_C3 = 0.1725851
EXPQ_C2 = 0.50206058
EXPQ_C1 = 0.99983348

# R(1): the approximate self-term for DVE-owned diagonal chunks
EXPQ_SELF = (1.0 + EXPQ_C1 + EXPQ_C2 + EXPQ_C3) ** 2

# chunk ownership: 17/32 to ACT, 15/32 to DVE, interleaved (Bresenham) so
# both engines consume concurrently under the 2-buffer PSUM pipeline.
N_ACT_CHUNKS = 17


def _act_owned(t: int, jc: int) -> bool:
    k = jc * NT + t
    return (k * N_ACT_CHUNKS) // 32 != ((k + 1) * N_ACT_CHUNKS) // 32


_CACHE: dict = {}


def _register_expq():
    for op in dve_ops.OPS:
        if op.name == EXPQ_NAME:
            return op
    q = ((C0 * Src0 + C1) * Src0 + C2) * Src0 + One
    spec = Spec(
        body=sq(q),
        accum=AluOp.ADD,
        reference=lambda in0, in1, s0, s1, imm2: (
            (((s0 * in0 + s1) * in0 + imm2) * in0 + 1.0) ** 2
        ),
    )
    row = dve_ops._CUSTOM_DVE_ROW_BASE + len(dve_ops.OPS)
    shas = {}
    for ver in ("v3", "v4"):
        comp = DveOpSpec(
            name=EXPQ_NAME, opcode=row, uops=lower(spec, ver=ver), rd1_en=False
        )
        shas[ver] = comp.sha(ver)
    op = dve_ops.DveOp(EXPQ_NAME, spec, subdim=False, uops_sha=shas)
    dve_ops.OPS.append(op)
    dve_ops._SUB_OPCODE_FOR_NAME[op.name] = row
    dve_ops.CUSTOM_DVE_SPECS[op.name] = op.spec
    return op


def _build_program() -> bass.Bass:
    expq = _register_expq()

    nc = bacc.Bacc(None)
    zt_in = nc.dram_tensor("zt", [D, TWO_B], FP8, kind="ExternalInput")
    partial = nc.dram_tensor("partial", [1, 1], F32, kind="ExternalOutput")

    NSUB = TWO_B // 512  # 16 x 512-col z tiles so matmuls start early

    with tile.TileContext(nc) as tc, ExitStack() as ctx:
        zp = ctx.enter_context(tc.tile_pool(name="zp", bufs=NSUB))
        pers = ctx.enter_context(tc.tile_pool(name="pers", bufs=1))
        psum = ctx.enter_context(tc.tile_pool(name="psum", bufs=2, space="PSUM"))

        zt = [zp.tile([D, 512], FP8, tag="zt", name=f"zt_{k}") for k in range(NSUB)]
        for k in range(NSUB):
            nc.sync.dma_start(out=zt[k], in_=zt_in[:, k * 512 : (k + 1) * 512])

        denA = pers.tile([128, NT, NJC], F32, tag="denA")
        denD = pers.tile([128, NT, NJC], F32, tag="denD")
        nc.vector.memset(denA, 0.0)
        nc.vector.memset(denD, 0.0)

        for jc in range(NJC):
            for t in range(NT):
                ch = psum.tile([128, NCHUNK], F32, tag="chunk")
                lhsT = zt[t // 4][:, (t % 4) * 128 : (t % 4 + 1) * 128]
                for a in range(4):
                    nc.tensor.matmul(
                        out=ch[:, a * 512 : (a + 1) * 512],
                        lhsT=lhsT,
                        rhs=zt[jc * 4 + a][:],
                        start=True,
                        stop=True,
                    )
                if _act_owned(t, jc):
                    nc.scalar.activation(
                        out=ch,
                        in_=ch,
                        func=AF.Exp,
                        scale=INV_T / SIMSCALE,
                        accum_out=denA[:, t, jc : jc + 1],
                    )
                else:
                    nc.vector._custom_dve(
                        expq,
                        out=ch,
                        in0=ch,
                        s0=EXPQ_C3 / SIMSCALE**3,
                        s1=EXPQ_C2 / SIMSCALE**2,
                        imm2=EXPQ_C1 / SIMSCALE,
                        accum_out=denD[:, t, jc : jc + 1],
                    )

        # tail: den8 = sum_jc denA + sum_jc denD - SELF_TERM, ln, row total
        dA = pers.tile([128, NT, 1], F32, tag="dA")
        dB = pers.tile([128, NT, 1], F32, tag="dB")
        nc.vector.reduce_sum(out=dA, in_=denA, axis=mybir.AxisListType.X)
        nc.vector.reduce_sum(out=dB, in_=denD, axis=mybir.AxisListType.X)
        den8 = pers.tile([128, NT], F32, tag="den8")
        nc.vector.tensor_add(den8, dA[:, :, 0], dB[:, :, 0])
        # subtract the per-row self term: diagonal chunk of row-block t is
        # (t, jc=0); its owner determines exact e^2 vs approx R(1)
        for t in range(NT):
            selfc = SELF_TERM if _act_owned(t, 0) else EXPQ_SELF
            nc.vector.tensor_scalar_add(
                den8[:, t : t + 1], den8[:, t : t + 1], -selfc
            )
        lnden = pers.tile([128, NT], F32, tag="lnden")
        nc.scalar.activation(out=lnden, in_=den8, func=AF.Ln)
        lr1 = pers.tile([128, 1], F32, tag="lr1")
        nc.vector.reduce_sum(out=lr1, in_=lnden, axis=mybir.AxisListType.X)
        ones = pers.tile([128, 1], F32, tag="ones")
        nc.vector.memset(ones, 1.0)
        fin = psum.tile([128, NCHUNK], F32, tag="chunk", name="fin")
        nc.tensor.matmul(
            out=fin[0:1, 0:1], lhsT=ones, rhs=lr1, start=True, stop=True
        )
        outsb = pers.tile([1, 1], F32, tag="outsb")
        nc.vector.tensor_copy(outsb, fin[0:1, 0:1])
        nc.sync.dma_start(out=partial[:], in_=outsb)

    nc.finalize()
    return nc


def _get_program() -> bass.Bass:
    if "nc" not in _CACHE:
        _CACHE["nc"] = _build_program()
    return _CACHE["nc"]


def _run(inputs: dict, trace: bool = False):
    import ml_dtypes

    nc = _get_program()
    emb_i = np.ascontiguousarray(inputs["emb_i"], dtype=np.float32)
    emb_j = np.ascontiguousarray(inputs["emb_j"], dtype=np.float32)
    eps = 1e-12
    z_i = emb_i / np.maximum(
        np.linalg.norm(emb_i, axis=1, keepdims=True), eps
    )
    z_j = emb_j / np.maximum(
        np.linalg.norm(emb_j, axis=1, keepdims=True), eps
    )
    pos_sum = float(np.einsum("bd,bd->", z_i, z_j, dtype=np.float64))
    # x16 pre-scale keeps fp8 e4m3 components in the normal range; the
    # device divides the resulting 256x sim scale back out.
    z = (16.0 * np.concatenate([z_i, z_j], axis=0)).astype(
        ml_dtypes.float8_e4m3fn
    )
    in_maps = [
        {"zt": np.ascontiguousarray(np.roll(z, -ROWS * c, axis=0).T)}
        for c in range(N_CORES)
    ]
    res = run_bass_kernel_spmd(nc, in_maps, list(range(N_CORES)), trace=trace)
    lnden_sum = sum(float(res.results[c]["partial"][0, 0]) for c in range(N_CORES))
    loss = (lnden_sum - 2.0 * INV_T * pos_sum) / TWO_B
    return np.float32(loss), res


def kernel(**inputs) -> np.ndarray:
    out, _ = _run(inputs)
    return np.asarray(out, dtype=np.float32)
